# revision 1
# baseline (speedup 1.0000x reference)
"""Trainium2 Bass kernel for Swin-style multi-head attention.

Problem: x[128,197,768] -> qkv -> 12-head attention with relative-position
bias -> proj. Data-parallel over batch across 8 NeuronCores (16 batches/core).

Dataflow (per core):
  - xT [16, 768, 197] fed pre-transposed from host; x and the qkv/proj
    weights are declared float32r on BOTH the DRAM and SBUF side (f32r is
    bit-identical fp32 in memory), so all loads are direct DMAs with no
    staging/conversion copies.
  - q,k computed feature-major via f32r matmuls (full-rate: free dim 394),
    then down-converted once: ACT copies PSUM -> bf16 staging, GPSIMD
    places them in the bf16 qk tile [128, 12, 2, 198].
  - v computed token-major the same way into a bf16 augmented "vaug"
    layout [t, 12, 66-alloc/65-used] whose 65th column is ones, so the AV
    matmul also emits the softmax row-sums for free.
  - scores (k^T q) and AV run in bf16 at 1 cycle/row with UNPADDED n=197
    free dims -- bf16 has no >=256 free-dim requirement, unlike f32r,
    which is worth ~19us/rep of PE time vs the padded-f32r variant.
  - softmax without max-subtraction; bias pre-gathered and pre-exponentiated
    on host: ACT does exp from PSUM into f32 sa, DVE multiplies by the bias
    factor into bf16 pu (mixed-dtype DVE op: deliberately NOT all-2-byte --
    all-bf16 DVE ops and strided bf16 ACT writes fault on this hardware).
  - O_unnorm + rowsum from one matmul; normalization via DVE reciprocal +
    GPSIMD partition_broadcast + DVE multiply; proj in f32r (rhs o_all
    [*, 394] free) + tensor_scalar bias + DMA out per e-tile.

Software pipeline: per pair, the 12 qk fills run first, then the 12
attention units (batch 0 then batch 1) are interleaved with the batch-1
V fills and the PREVIOUS pair's 6 projection units as PE filler work, so
the PE does not idle while ACT/DVE work through each unit's softmax
epilogue (exp -> bias-mult -> AV, ~2.5us latency). AV emission lags
scores by pdepth=4 units. qk tiles are double-buffered so next-pair fills
never wait on the last scores reads; x for pair p+1 is DMA-prefetched
mid-pair. PSUM budget: 2 fill banks + 4 score banks + 2 AV banks = 8.
"""

import sys

import numpy as np

for _p in ('/opt/trn_rl_repo', '/root/.axon_site/_ro/trn_rl_repo'):
    if _p not in sys.path:
        sys.path.insert(0, _p)

B = 128
N = 197
NPAD = 256
C = 768
H = 12
DH = 64
SCALE = DH ** -0.5
NCORES = 8
BLOC = B // NCORES  # 16
M0, M1 = 128, N - 128  # key-dim tiles: 128 + 69


def build_nc(b_loc=BLOC, lin_r=True, attn_r=True, pdepth=4, reps=1):
    """Build the per-core Bass program.

    lin_r / attn_r: use float32r (full-rate single-pass fp32) for the
    qkv+proj / attention matmuls respectively.
    """
    import concourse.bacc as bacc
    import concourse.tile as tile
    from concourse import library_config, mybir

    f32 = mybir.dt.float32
    f32r = mybir.dt.float32r
    abf = mybir.dt.bfloat16
    N2 = N + 1  # 198: bf16 tiles padded so all strides stay 4B-aligned
    lin_dt = f32r if lin_r else f32
    attn_dt = f32r if attn_r else f32

    nc = bacc.Bacc("TRN2", target_bir_lowering=False, debug=False)
    # f32r is bit-identical fp32 in memory; declaring the DRAM side f32r
    # lets the DMA land directly in f32r SBUF tiles (no staging copies).
    xT = nc.dram_tensor("xT", [b_loc, C, N], lin_dt, kind="ExternalInput").ap()
    qkv_wT = nc.dram_tensor("qkv_wT", [C, 3 * C], lin_dt,
                            kind="ExternalInput").ap()
    proj_wT = nc.dram_tensor("proj_wT", [C, C], lin_dt,
                             kind="ExternalInput").ap()
    proj_bt = nc.dram_tensor("proj_bt", [128, 6], f32, kind="ExternalInput").ap()
    biasT = nc.dram_tensor("biasT", [H, N, NPAD], f32, kind="ExternalInput").ap()
    outT = nc.dram_tensor("outT", [b_loc, C, N], f32, kind="ExternalOutput").ap()

    n_pairs = b_loc // 2

    with tile.TileContext(nc) as tc:
        with (
            tc.tile_pool(name="consts", bufs=1) as consts,
            tc.tile_pool(name="xtp", bufs=1) as xtp,
            tc.tile_pool(name="qkp", bufs=2) as qkp,
            tc.tile_pool(name="vtp", bufs=1) as vtp,
            tc.tile_pool(name="sap", bufs=2) as sap,
            tc.tile_pool(name="pup", bufs=pdepth + 1) as pup,
            tc.tile_pool(name="recp", bufs=2) as recp,
            tc.tile_pool(name="oallp", bufs=2) as oallp,
            tc.tile_pool(name="obp", bufs=2) as obp,
            tc.tile_pool(name="psbig", bufs=2, space="PSUM") as psbig,
            tc.tile_pool(name="pss", bufs=2, space="PSUM") as pss,
            tc.tile_pool(name="pso", bufs=2, space="PSUM") as pso,
        ):
            nc.gpsimd.load_library(library_config.attnmlp)

            qkvw_sb = consts.tile([128, 6, 3 * C], lin_dt)
            projw_sb = consts.tile([128, 6, C], lin_dt)
            projb_sb = consts.tile([128, 6], f32)
            bias0_sb = consts.tile([128, H, N], f32)
            bias1_sb = consts.tile([128, H, N], f32)
            bias_sb = (bias0_sb, bias1_sb)
            zeros_sb = consts.tile([128, NPAD - N], f32)
            ones_sb = consts.tile([128, H], mybir.dt.bfloat16)

            def load_consts():
                nc.sync.dma_start(
                    qkvw_sb, qkv_wT.rearrange("(ct p) f -> p ct f", p=128))
                nc.sync.dma_start(
                    projw_sb, proj_wT.rearrange("(hp p) e -> p hp e", p=128))
                nc.sync.dma_start(projb_sb, proj_bt)
                nc.sync.dma_start(
                    bias0_sb, biasT[:, 0:M0, :N].rearrange("h p n -> p h n"))
                nc.sync.dma_start(
                    bias1_sb[:M1], biasT[:, M0:N, :N].rearrange("h p n -> p h n"))
                nc.vector.memset(zeros_sb, 0.0)
                nc.vector.memset(ones_sb, 1.0)

            load_consts()

            def dma_xt(pp):
                b0_ = 2 * (pp % n_pairs)
                xt = xtp.tile([128, 6, 2, N], lin_dt, tag="xt",
                              name=f"xt{pp}")
                for b in (0, 1):
                    nc.sync.dma_start(
                        xt[:, :, b, :],
                        xT[b0_ + b].rearrange("(ct p) n -> p ct n", p=128),
                    )
                return xt

            total = reps * n_pairs
            prev_proj_units = []
            xt_cur = dma_xt(0)
            xt_next = None
            for pp in range(total):
                b0 = 2 * (pp % n_pairs)
                xt = xt_cur

                # ---- q/k feature-major bf16 [f-tile, b, n] (no padding:
                # bf16 matmuls have no >=256 free-dim requirement) ----
                qk = qkp.tile([128, H, 2, N2], abf, tag="qk", name=f"qk{pp}")
                for ft in range(12):
                    ps = psbig.tile([128, 2, N], f32, tag="mmbig")
                    for ct in range(6):
                        nc.tensor.matmul(
                            ps,
                            qkvw_sb[:, ct, ft * 128:(ft + 1) * 128],
                            xt[:, ct],
                            start=(ct == 0),
                            stop=(ct == 5),
                        )
                    qst = obp.tile([128, 2, N], abf, tag="qst")
                    nc.scalar.copy(out=qst, in_=ps)
                    nc.gpsimd.tensor_copy(out=qk[:, ft, :, :N], in_=qst)

                # ---- v token-major, augmented layout [t, 12, 65] ----
                vts = [[None, None], [None, None]]

                def vfill(b, tci, half):
                    t0, tsz = ((0, M0), (M0, M1))[tci]
                    vt = vts[b][tci]
                    vt_r = vt.rearrange("p (g two) c -> p two g c", two=2)
                    psv = psbig.tile([128, 384], f32, tag="mmbig")
                    for ct in range(6):
                        nc.tensor.matmul(
                            psv[:tsz],
                            xt[:, ct, b, t0:t0 + tsz],
                            qkvw_sb[:, ct, 2 * C + half * 384:2 * C + (half + 1) * 384],
                            start=(ct == 0),
                            stop=(ct == 5),
                        )
                    vst = obp.tile([128, 384], abf, tag="vst")
                    nc.scalar.copy(out=vst[:tsz], in_=psv[:tsz])
                    vst_r = vst.rearrange("p (g two d) -> p two g d",
                                          two=2, d=64)
                    for par in (0, 1):
                        nc.gpsimd.tensor_copy(
                            out=vt_r[:tsz, par, half * 3:(half + 1) * 3, 0:64],
                            in_=vst_r[:tsz, par],
                        )
                    if half == 1:
                        nc.gpsimd.tensor_copy(out=vt[:tsz, :, 64],
                                              in_=ones_sb[:tsz])

                for b in (0, 1):
                    for tci in (0, 1):
                        # 66-col alloc keeps bf16 head-stride 4B-aligned;
                        # AV reads cols 0:65 only.
                        vts[b][tci] = vtp.tile([128, H, 66], abf,
                                               tag=f"vt{b}{tci}",
                                               name=f"vt{b}{tci}")
                # b=0 V tiles are needed soon (first AV); emit them now.
                for tci in (0, 1):
                    for half in (0, 1):
                        vfill(0, tci, half)

                # ---- attention, software-pipelined over (batch, head-pair) ----
                o_all = oallp.tile([128, 6, 2, N], lin_dt)

                def emit_scores(b, hp):
                    """scores + bias + exp for both heads of pair hp; returns pu tiles."""
                    h0, h1 = 2 * hp, 2 * hp + 1
                    q0 = qk[0:64, hp, b, :N]
                    k0 = qk[0:64, 6 + hp, b, :N]
                    q1 = qk[64:128, hp, b, :N]
                    k1 = qk[64:128, 6 + hp, b, :N]
                    pus = []
                    for mt, (m0, msz) in enumerate(((0, M0), (M0, M1))):
                        ps_e = pss.tile([128, NPAD], f32, tag="se")
                        ps_o = pss.tile([128, NPAD], f32, tag="so")
                        nc.tensor.matmul(
                            ps_e[:msz, :N], k0[:, m0:m0 + msz], q0,
                            start=True, stop=True,
                        )
                        nc.tensor.matmul(
                            ps_o[:msz, :N], k1[:, m0:m0 + msz], q1,
                            start=True, stop=True,
                        )
                        sa_pair = sap.tile([128, 2, N], f32, tag="sa")
                        pu_pair = pup.tile([128, 2, N2], abf, tag=f"pu{mt}")
                        nc.scalar.activation(
                            out=sa_pair[:msz, 0, :], in_=ps_e[:msz, :N],
                            func=mybir.ActivationFunctionType.Exp, scale=SCALE,
                        )
                        nc.scalar.activation(
                            out=sa_pair[:msz, 1, :], in_=ps_o[:msz, :N],
                            func=mybir.ActivationFunctionType.Exp, scale=SCALE,
                        )
                        nc.vector.tensor_mul(
                            out=pu_pair[:msz, :, :N], in0=sa_pair[:msz],
                            in1=bias_sb[mt][:msz, h0:h0 + 2, :],
                        )
                        pus.append(pu_pair)
                    return pus

                def emit_av(b, hp, pus):
                    h0, h1 = 2 * hp, 2 * hp + 1
                    vt0, vt1 = vts[b]
                    ps_pair = pso.tile([128, 2, NPAD], f32, tag="opair")
                    for par, h in ((0, h0), (1, h1)):
                        nc.tensor.matmul(
                            ps_pair[0:65, par, :N], vt0[:, h, 0:65],
                            pus[0][:, par, :N], start=True, stop=False,
                        )
                        nc.tensor.matmul(
                            ps_pair[0:65, par, :N], vt1[:M1, h, 0:65],
                            pus[1][:M1, par, :N], start=False, stop=True,
                        )
                    rec_pair = recp.tile([1, 2, N], f32, tag="rec")
                    nc.vector.reciprocal(out=rec_pair,
                                         in_=ps_pair[64:65, :, :N])
                    recb_pair = recp.tile([64, 2, N], f32, tag="recb")
                    for par in (0, 1):
                        nc.gpsimd.partition_broadcast(
                            recb_pair[:, par, :], rec_pair[:, par, :N]
                        )
                        nc.vector.tensor_mul(
                            out=o_all[par * 64:par * 64 + 64, hp, b, :],
                            in0=ps_pair[0:64, par, :N],
                            in1=recb_pair[:, par, :],
                        )

                def make_proj_unit(et, o_all_=o_all, b0_=b0):
                    def unit():
                        psp = psbig.tile([128, 2, N], f32, tag="mmbig", name=f"psp{et}")
                        for hp in range(6):
                            nc.tensor.matmul(
                                psp,
                                projw_sb[:, hp, et * 128:(et + 1) * 128],
                                o_all_[:, hp],
                                start=(hp == 0),
                                stop=(hp == 5),
                            )
                        ob = obp.tile([128, 2, N], f32, tag="ob", name=f"ob{et}")
                        nc.vector.tensor_scalar_add(ob, psp, projb_sb[:, et:et + 1])
                        nc.sync.dma_start(
                            outT[b0_:b0_ + 2, et * 128:(et + 1) * 128, :].rearrange(
                                "b p n -> p b n"
                            ),
                            ob,
                        )
                    return unit

                fillers = [lambda tci=tci, half=half: vfill(1, tci, half)
                           for tci in (0, 1) for half in (0, 1)]
                fillers += prev_proj_units
                work = [(b, hp) for b in (0, 1) for hp in range(6)]
                pending = []
                for i, (b, hp) in enumerate(work):
                    pending.append((b, hp, emit_scores(b, hp)))
                    if fillers:
                        fillers.pop(0)()
                    if len(pending) > pdepth:
                        pb, php, ppus = pending.pop(0)
                        emit_av(pb, php, ppus)
                if pp + 1 < total:
                    xt_next = dma_xt(pp + 1)
                for pb, php, ppus in pending:
                    emit_av(pb, php, ppus)
                for u in fillers:
                    u()
                prev_proj_units = [make_proj_unit(et) for et in range(6)]
                xt_cur = xt_next

            # ---- final pair's proj ----
            for u in prev_proj_units:
                u()
    nc.compile()
    return nc


def prep_inputs(x, qkv_w, proj_w, proj_b, bias_table, rel_idx):
    """Host-side data prep shared by kernel() and test harness."""
    x = np.asarray(x, np.float32)
    qkv_w = np.asarray(qkv_w, np.float32)
    proj_w = np.asarray(proj_w, np.float32)
    proj_b = np.asarray(proj_b, np.float32)
    bias_table = np.asarray(bias_table, np.float32)
    rel_idx = np.asarray(rel_idx)

    xT = np.ascontiguousarray(x.reshape(NCORES, BLOC, N, C).transpose(0, 1, 3, 2))
    qkv_wT = np.ascontiguousarray(qkv_w.T)
    proj_wT = np.ascontiguousarray(proj_w.T)
    proj_bt = np.ascontiguousarray(proj_b.reshape(6, 128).T)
    bias_full = bias_table[rel_idx]  # [n, m, h]
    biasT = np.ones((H, N, NPAD), np.float32)
    biasT[:, :, :N] = np.exp(bias_full.transpose(2, 1, 0))
    return xT, qkv_wT, proj_wT, proj_bt, biasT


def make_in_maps(x, qkv_w, proj_w, proj_b, bias_table, rel_idx):
    xT, qkv_wT, proj_wT, proj_bt, biasT = prep_inputs(
        x, qkv_w, proj_w, proj_b, bias_table, rel_idx
    )
    return [
        {
            "xT": np.ascontiguousarray(xT[c]),
            "qkv_wT": qkv_wT,
            "proj_wT": proj_wT,
            "proj_bt": proj_bt,
            "biasT": biasT,
        }
        for c in range(NCORES)
    ]


def assemble_out(res):
    outs = np.stack([res.results[c]["outT"] for c in range(NCORES)])  # [8,16,768,197]
    out = outs.reshape(B, C, N).transpose(0, 2, 1)
    return np.ascontiguousarray(out, np.float32)


_NC_CACHE = {}


def _get_nc(**kw):
    key = tuple(sorted(kw.items()))
    if key not in _NC_CACHE:
        _NC_CACHE[key] = build_nc(**kw)
    return _NC_CACHE[key]


def kernel(x, qkv_w, proj_w, proj_b, bias_table, rel_idx,
           _lin_r=True, _attn_r=True, _trace=False):
    from concourse.bass_utils import run_bass_kernel_spmd

    in_maps = make_in_maps(x, qkv_w, proj_w, proj_b, bias_table, rel_idx)
    nc = _get_nc(lin_r=_lin_r, attn_r=_attn_r)
    res = run_bass_kernel_spmd(nc, in_maps, list(range(NCORES)), trace=_trace)
    out = assemble_out(res)
    if _trace:
        return out, res
    return out

